# revision 4
# baseline (speedup 1.0000x reference)
# Trainium2 Bass kernel for nn_Invo2D (involution-style dynamic conv).
#
# Math:
#   t2 = x @ (W1@W2) + (b1@W2 + b2)            [pix, 144]   (both 1x1 convs are linear)
#   P[pix, f] = 3x3 SAME patches, f = ki*256 + ch
#   out[pix, co] = sum_j t2[pix, 9*(co//16)+j] * P[pix, 9*co+j]
#
# Sharding: data-parallel over batch, 1 image per core (8 cores).
#
# Layout: partition p = 64*wq + h; per-partition free = 34 w-slots x 256 ch
# (slot s <-> w = 32*wq + s - 1, halo zeroed on host). Host uploads bf16:
#  - 6 pixel-major x views: {row-1, row, row+1} x {even, odd(+1 elem shift)}
#    so every multiply AP is 4B-aligned step-1 (2x DVE packing)
#  - channel-major xcm for the weight matmul
# Products go to 16-padded (co,slot) blocks; fold = 9 identity-matmul
# accumulates into PSUM f32 on the tensor engine (or DVE tree fallback).

import numpy as np
import ml_dtypes

H, W, C = 64, 64, 256
G, GC = 16, 16
M144, D = 144, 64
NCORES = 8
SLOTS = 34
XF = SLOTS * C + 2      # 8706 (2 slack for odd-shifted views)
WLC = 8                 # wl per chunk
NCHUNK = 4
PF = WLC * 2048         # per-half product free size (8 wl x 128 co x 16 slots)

FOLD_ON_PE = False

_cache = {}


def _rect_decomp(r0, r1):
    out = []
    gc0, j0 = divmod(r0, 9)
    if j0 != 0:
        end = min(r1, (gc0 + 1) * 9)
        out.append((gc0, 1, j0, end - r0))
        r0 = end
        if r0 == r1:
            return out
        gc0, j0 = divmod(r0, 9)
    nfull = (r1 - r0) // 9
    if nfull:
        out.append((gc0, nfull, 0, 9))
        r0 += nfull * 9
        gc0 += nfull
    if r0 < r1:
        out.append((gc0, 1, 0, r1 - r0))
    return out


def _build_instrs():
    """Parity-aligned multiply instructions.
    Each: (g, ki, gc0, gstep, ngc, jlo, njr, xpar)."""
    instrs = []
    for g in range(16):
        f_lo, f_hi = 144 * g, 144 * g + 144
        cuts = [f_lo] + [256 * k for k in range(1, 9) if f_lo < 256 * k < f_hi] + [f_hi]
        for a, b in zip(cuts, cuts[1:]):
            ki = a // 256
            for (gc0, ngc, j0, nj) in _rect_decomp(a - 144 * g, b - 144 * g):
                if ngc == 1:
                    halves = [(gc0, 1, 1)]
                else:
                    halves = [(gc0, 2, (ngc + 1) // 2), (gc0 + 1, 2, ngc // 2)]
                    halves = [h for h in halves if h[2] > 0]
                for (gcs, step, n) in halves:
                    ch0 = 144 * g + 9 * gcs + j0 - 256 * ki
                    # left-extend odd-j0 runs by one (duplicate of the j0-1
                    # product) ONLY when it stays within this tap's channels
                    ext = 1 if (j0 % 2 == 1 and ch0 >= 1) else 0
                    xpar = (ch0 - ext) % 2
                    instrs.append((g, ki, gcs, step, n, j0 - ext, nj + ext, xpar))
    return instrs


def _build_program():
    import concourse.bass as bass
    import concourse.tile as tile
    from concourse import bacc, mybir
    from concourse.masks import make_identity

    f32 = mybir.dt.float32
    bf16 = mybir.dt.bfloat16
    AP = bass.AP

    nc = bacc.Bacc(None, target_bir_lowering=False)
    xball_d = nc.dram_tensor("xball", [128, NCHUNK * 6 * 2562], bf16,
                             kind="ExternalInput")
    xcm_d = nc.dram_tensor("xcm", [128, NCHUNK * 2048], bf16, kind="ExternalInput")
    w12_d = nc.dram_tensor("w12", [2, 128, M144], bf16, kind="ExternalInput")
    brow_d = nc.dram_tensor("brow", [1, M144], bf16, kind="ExternalInput")
    out_d = nc.dram_tensor("out", [128, 32 * 256], bf16, kind="ExternalOutput")

    INSTRS = _build_instrs()

    with tile.TileContext(nc) as tc:
        with (
            tc.tile_pool(name="consts", bufs=1) as consts,
            tc.tile_pool(name="xb", bufs=1) as xbp,
            tc.tile_pool(name="xcm", bufs=1) as xcmp,
            tc.tile_pool(name="w16", bufs=1) as w16p,
            tc.tile_pool(name="prod", bufs=1) as prodp,
            tc.tile_pool(name="outs", bufs=1) as outsp,
            tc.tile_pool(name="ps2", bufs=3, space="PSUM") as ps2p,
            tc.tile_pool(name="pswu", bufs=1, space="PSUM") as pswup,
            tc.tile_pool(name="psr", bufs=2, space="PSUM") as psrp,
        ):
            ident = consts.tile([128, 128], bf16)
            make_identity(nc, ident[:])
            # preload the ACT copy table set early (one tiny copy)
            nc.scalar.copy(out=ident[0:1, 0:2], in_=ident[0:1, 0:2])
            w12a = consts.tile([128, M144], bf16)
            w12b = consts.tile([128, M144], bf16)
            
            nc.sync.dma_start(out=w12a[:], in_=w12_d[0])
            nc.sync.dma_start(out=w12b[:], in_=w12_d[1])
            brow = consts.tile([1, M144], bf16)
            nc.sync.dma_start(out=brow[:], in_=brow_d[:])
            ones1 = consts.tile([1, 128], bf16)
            nc.gpsimd.memset(ones1[:], 1.0)

            pswu = pswup.tile([128, 128], f32)
            for _ in range(30):
                nc.tensor.matmul(pswu[:], lhsT=ident[:], rhs=ident[:],
                                 start=True, stop=True)
            def dma_xcm_chunk(c):
                t = xcmp.tile([128, 2048], bf16, name=f"xcm_{c % 2}")
                nc.sync.dma_start(
                    out=t[:],
                    in_=AP(xcm_d, c * 2048, [[NCHUNK * 2048, 128], [1, 2048]]))
                return t

            # x views: one packed per-chunk tile (6 views x 2562), 2-deep
            XNAMES = ("x0e", "x0o", "xue", "xuo", "xde", "xdo")
            XIDX = {n: i for i, n in enumerate(XNAMES)}
            XCH = 10 * 256 + 2

            def dma_x_chunk(c):
                t = xbp.tile([128, 6 * XCH], bf16, name=f"xpk_{c % 2}")
                for h in range(2):
                    nc.sync.dma_start(
                        out=AP(t.tensor, h * 3 * XCH, [[6 * XCH, 128], [1, 3 * XCH]]),
                        in_=AP(xball_d, c * 6 * XCH + h * 3 * XCH,
                               [[NCHUNK * 6 * XCH, 128], [1, 3 * XCH]]),
                    )
                return t

            def t2gen(c, xcmt, w16t):
                # 8 tiles: wl = 8c+t ; ps2[pix,144] = xcm_tile.T @ W12 + bias
                for t in range(WLC):
                    ps2 = ps2p.tile([128, M144], f32)
                    nc.tensor.matmul(
                        ps2[:],
                        lhsT=AP(xcmt.tensor, t * 128, [[2048, 128], [1, 128]]),
                        rhs=w12a[:], start=True, stop=False)
                    nc.tensor.matmul(
                        ps2[:],
                        lhsT=AP(xcmt.tensor, 1024 + t * 128, [[2048, 128], [1, 128]]),
                        rhs=w12b[:], start=False, stop=False)
                    nc.tensor.matmul(ps2[:], lhsT=ones1[:], rhs=brow[:],
                                     start=False, stop=True)
                    # scatter t2[9g+j] -> W16[t*256 + 16g+j]
                    nc.scalar.copy(
                        out=AP(w16t.tensor, t * 256,
                               [[2048, 128], [16, 16], [1, 9]]),
                        in_=AP(ps2.tensor, 0, [[M144, 128], [9, 16], [1, 9]]),
                    )

            XNAME = {(-1, 0): "xde", (-1, 1): "xdo", (0, 0): "x0e",
                     (0, 1): "x0o", (1, 0): "xue", (1, 1): "xuo"}

            def mults(c, xt, w16t, Ma, Mb, want_b):
                    for (g, ki, gcs, step, n, jlo, njr, xpar) in INSTRS:
                        if (g >= 8) != want_b:
                            continue
                        di, dj = ki // 3 - 1, ki % 3 - 1
                        voff = XIDX[XNAME[(di, xpar)]] * XCH
                        xb = xt
                        ch0 = 144 * g + 9 * gcs + jlo - 256 * ki + xpar + voff
                        Mx = Mb if g >= 8 else Ma
                        col = (16 * g + gcs) - (128 if g >= 8 else 0)
                        in_dims = [[6 * XCH, 128], [256, WLC]]
                        w_dims = [[2048, 128], [256, WLC]]
                        o_dims = [[PF // WLC * WLC, 128], [2048, WLC]]
                        if n > 1:
                            in_dims.append([9 * step, n])
                            w_dims.append([0, n])
                            o_dims.append([16 * step, n])
                        in_dims.append([1, njr])
                        w_dims.append([1, njr])
                        o_dims.append([1, njr])
                        nc.vector.tensor_mul(
                            AP(Mx.tensor, col * 16 + jlo, o_dims),
                            AP(xb.tensor, (dj + 1) * 256 + ch0, in_dims),
                            AP(w16t.tensor, 16 * g + jlo, w_dims),
                        )

            def fold_pe(c, Mx, half, outc):
                # 2 groups of 512 (4 wl x 128 co); 9 slot-plane accum-MMs each
                for k in range(2):
                    psf = psfp.tile([128, 512], f32)
                    for s in range(9):
                        nc.tensor.matmul(
                            psf[:],
                            lhsT=ident[:],
                            rhs=AP(Mx.tensor, 4 * k * 2048 + s,
                                   [[PF, 128], [2048, 4], [16, 128]]),
                            start=(s == 0), stop=(s == 8))
                    # outc free layout: [wl(8), co(256)]; half b at col 128
                    nc.scalar.copy(
                        out=AP(outc.tensor, (4 * k) * 256 + half * 128,
                               [[2048, 128], [256, 4], [1, 128]]),
                        in_=AP(psf.tensor, 0, [[512, 128], [128, 4], [1, 128]]),
                    )

            def fold_dve(c, Mx, half, outc):
                # F1 in place: M[4..7] += M[0..3]
                nc.vector.tensor_add(
                    AP(Mx.tensor, 4, [[PF, 128], [2048, WLC], [16, 128], [1, 4]]),
                    AP(Mx.tensor, 0, [[PF, 128], [2048, WLC], [16, 128], [1, 4]]),
                    AP(Mx.tensor, 4, [[PF, 128], [2048, WLC], [16, 128], [1, 4]]),
                )
                # PE: psf = planes 4..8 per 512-group (4 wl x 128 co)
                for k in range(2):
                    psr = psrp.tile([128, 512], f32)
                    for si, sl in enumerate((4, 5, 6, 7, 8)):
                        nc.tensor.matmul(
                            psr[:], lhsT=ident[:],
                            rhs=AP(Mx.tensor, 4 * k * 2048 + sl,
                                   [[PF, 128], [2048, 4], [16, 128]]),
                            start=(si == 0), stop=(si == 4))
                    nc.scalar.copy(
                        out=AP(outc.tensor, (4 * k) * 256 + half * 128,
                               [[2048, 128], [256, 4], [1, 128]]),
                        in_=AP(psr.tensor, 0, [[512, 128], [128, 4], [1, 128]]),
                    )

            W16t = [w16p.tile([128, WLC * 256], bf16, name=f"w16_{i}") for i in range(4)]
            Ma = prodp.tile([128, PF], bf16)
            Mb = prodp.tile([128, PF], bf16)
            OUTt = [outsp.tile([128, WLC * 256], bf16, name=f"out_{i}") for i in range(2)]

            xcms = {0: dma_xcm_chunk(0)}
            xts = {0: dma_x_chunk(0), 1: dma_x_chunk(1)}
            t2gen(0, xcms.pop(0), W16t[0])
            xcms[1] = dma_xcm_chunk(1)
            for c in range(NCHUNK):
                xt = xts.pop(c)
                outc = OUTt[c % 2]
                mults(c, xt, W16t[c], Ma, Mb, False)
                fold_dve(c, Ma, 0, outc)
                nc.sync.dma_start(
                    out=AP(out_d, c * WLC * 256, [[32 * 256, 128], [256, WLC], [1, 128]]),
                    in_=AP(outc.tensor, 0, [[WLC * 256, 128], [256, WLC], [1, 128]]),
                )
                if c + 1 < NCHUNK:
                    t2gen(c + 1, xcms.pop(c + 1), W16t[c + 1])
                if c + 2 < NCHUNK:
                    xcms[c + 2] = dma_xcm_chunk(c + 2)
                if c == NCHUNK - 1:
                    for _ in range(25):
                        nc.tensor.matmul(pswu[:], lhsT=ident[:], rhs=ident[:],
                                         start=True, stop=True)
                mults(c, xt, W16t[c], Ma, Mb, True)
                fold_dve(c, Mb, 1, outc)
                if c + 2 < NCHUNK:
                    xts[c + 2] = dma_x_chunk(c + 2)
                nc.sync.dma_start(
                    out=AP(out_d, c * WLC * 256 + 128,
                           [[32 * 256, 128], [256, WLC], [1, 128]]),
                    in_=AP(outc.tensor, 128, [[WLC * 256, 128], [256, WLC], [1, 128]]),
                )
    nc.compile()
    return nc


def _get_program():
    if "nc" not in _cache:
        _cache["nc"] = _build_program()
    return _cache["nc"]


XCH2 = 10 * 256 + 2


def _host_prep(x, W1, b1, W2, b2):
    bf = ml_dtypes.bfloat16
    W12 = (W1.astype(np.float64) @ W2.astype(np.float64)).astype(np.float32)
    bfused = (b1.astype(np.float64) @ W2.astype(np.float64) + b2).astype(np.float32)
    w12_h = np.ascontiguousarray(W12.astype(bf).reshape(2, 128, M144))
    brow_h = np.ascontiguousarray(bfused.astype(bf).reshape(1, M144))

    in_maps = []
    for i in range(x.shape[0]):
        xi = x[i].astype(bf)  # [64, 64, 256]
        # pixel-major: [p=64*wq+h, slot, ch]; slot s <-> w = 32*wq + s - 1
        xpm4 = np.zeros((2, 64, SLOTS, 256), dtype=bf)
        xpm4[0, :, 1:34, :] = xi[:, 0:33, :]
        xpm4[1, :, 0:33, :] = xi[:, 31:64, :]
        xpm = np.zeros((128, XF), dtype=bf)
        xpm[:, :SLOTS * 256] = xpm4.reshape(128, SLOTS * 256)
        xu = np.zeros_like(xpm)   # row h+1
        xd = np.zeros_like(xpm)   # row h-1
        for wq in range(2):
            xu[64 * wq:64 * wq + 63] = xpm[64 * wq + 1:64 * wq + 64]
            xd[64 * wq + 1:64 * wq + 64] = xpm[64 * wq:64 * wq + 63]
        def odd(a):
            o = np.zeros_like(a)
            o[:, 1:] = a[:, :-1]
            return o
        # channel-major packed per chunk: [ch%128, c, ch//128, wl_t*128+p]
        xc = xi.reshape(64, 2, 32, 256).transpose(3, 2, 1, 0).reshape(2, 128, 32, 128)
        # xc[h, p, wl, pix]; want [p, chunk, h, t*128+pix]
        xcm = np.ascontiguousarray(
            xc.reshape(2, 128, 4, 8, 128).transpose(1, 2, 0, 3, 4).reshape(128, -1))
        views = [xpm, odd(xpm), xu, odd(xu), xd, odd(xd)]
        xball = np.zeros((128, NCHUNK, 6, XCH2), dtype=bf)
        for c in range(NCHUNK):
            for v, a in enumerate(views):
                xball[:, c, v, :] = a[:, WLC * c * 256:WLC * c * 256 + XCH2]
        in_maps.append({
            "xball": np.ascontiguousarray(xball.reshape(128, -1)),
            "xcm": xcm, "w12": w12_h, "brow": brow_h,
        })
    return in_maps


def kernel(x, W1, b1, W2, b2, trace=False):
    from concourse.bass_utils import run_bass_kernel_spmd

    nc = _get_program()
    in_maps = _host_prep(np.asarray(x), W1, b1, W2, b2)
    res = run_bass_kernel_spmd(nc, in_maps, core_ids=list(range(NCORES)),
                               trace=trace)
    outs = []
    for i in range(NCORES):
        buf = np.asarray(res.results[i]["out"]).astype(np.float32)
        # [64*wq+h, wl*256+c] -> [h, w, c]
        o = buf.reshape(2, 64, 32, 256).transpose(1, 0, 2, 3).reshape(64, 64, 256)
        outs.append(o)
    out = np.stack(outs, axis=0)
    if trace:
        return out, res
    return out


# revision 11
# speedup vs baseline: 1.2996x; 1.2996x over previous
# Trainium2 Bass kernel for nn_Invo2D (involution-style dynamic conv).
#
# Math:
#   t2 = x @ (W1@W2) + (b1@W2 + b2)            [pix, 144]   (both 1x1 convs are linear)
#   P[pix, f] = 3x3 SAME patches, f = ki*256 + ch
#   out[pix, co] = sum_j t2[pix, 9*(co//16)+j] * P[pix, 9*co+j]
#
# Sharding: data-parallel over batch, 1 image per core (8 cores).
#
# Layout: partition p = 64*wq + h; per-partition free = 34 w-slots x 256 ch
# (slot s <-> w = 32*wq + s - 1, halo zeroed on host). Host uploads bf16:
#  - 6 pixel-major x views: {row-1, row, row+1} x {even, odd(+1 elem shift)}
#    so every multiply AP is 4B-aligned step-1 (2x DVE packing)
#  - channel-major xcm for the weight matmul
# Products go to 16-padded (co,slot) blocks; fold = 9 identity-matmul
# accumulates into PSUM f32 on the tensor engine (or DVE tree fallback).

import numpy as np
import ml_dtypes

H, W, C = 64, 64, 256
G, GC = 16, 16
M144, D = 144, 64
NCORES = 8
SLOTS = 34
XF = SLOTS * C + 2      # 8706 (2 slack for odd-shifted views)
WLC = 8                 # wl per chunk
NCHUNK = 4
PF = WLC * 2048         # per-half product free size (8 wl x 128 co x 16 slots)

FOLD_ON_PE = False

_cache = {}


def _rect_decomp(r0, r1):
    out = []
    gc0, j0 = divmod(r0, 9)
    if j0 != 0:
        end = min(r1, (gc0 + 1) * 9)
        out.append((gc0, 1, j0, end - r0))
        r0 = end
        if r0 == r1:
            return out
        gc0, j0 = divmod(r0, 9)
    nfull = (r1 - r0) // 9
    if nfull:
        out.append((gc0, nfull, 0, 9))
        r0 += nfull * 9
        gc0 += nfull
    if r0 < r1:
        out.append((gc0, 1, 0, r1 - r0))
    return out


def _build_instrs():
    """Parity-aligned multiply instructions.
    Each: (g, ki, gc0, gstep, ngc, jlo, njr, xpar)."""
    instrs = []
    for g in range(16):
        f_lo, f_hi = 144 * g, 144 * g + 144
        cuts = [f_lo] + [256 * k for k in range(1, 9) if f_lo < 256 * k < f_hi] + [f_hi]
        for a, b in zip(cuts, cuts[1:]):
            ki = a // 256
            for (gc0, ngc, j0, nj) in _rect_decomp(a - 144 * g, b - 144 * g):
                if ngc == 1:
                    halves = [(gc0, 1, 1)]
                else:
                    halves = [(gc0, 2, (ngc + 1) // 2), (gc0 + 1, 2, ngc // 2)]
                    halves = [h for h in halves if h[2] > 0]
                for (gcs, step, n) in halves:
                    ch0 = 144 * g + 9 * gcs + j0 - 256 * ki
                    # left-extend odd-j0 runs by one (duplicate of the j0-1
                    # product) ONLY when it stays within this tap's channels
                    ext = 1 if (j0 % 2 == 1 and ch0 >= 1) else 0
                    xpar = (ch0 - ext) % 2
                    instrs.append((g, ki, gcs, step, n, j0 - ext, nj + ext, xpar))
    return instrs


def _build_program(bias_zero=False):
    import concourse.bass as bass
    import concourse.tile as tile
    from concourse import bacc, mybir
    from concourse.masks import make_identity

    f32 = mybir.dt.float32
    bf16 = mybir.dt.bfloat16
    AP = bass.AP

    nc = bacc.Bacc(None, target_bir_lowering=False)
    xball_d = nc.dram_tensor("xball", [128, NCHUNK * 6 * 2562], bf16,
                             kind="ExternalInput")
    xcm_d = nc.dram_tensor("xcm", [128, NCHUNK * 2048], bf16, kind="ExternalInput")
    w12_d = nc.dram_tensor("w12", [2, 128, M144], bf16, kind="ExternalInput")
    brow_d = nc.dram_tensor("brow", [1, M144], bf16, kind="ExternalInput")
    out_d = nc.dram_tensor("out", [128, 32 * 256], bf16, kind="ExternalOutput")

    INSTRS = _build_instrs()

    with tile.TileContext(nc) as tc:
        with (
            tc.tile_pool(name="consts", bufs=1) as consts,
            tc.tile_pool(name="xb", bufs=1) as xbp,
            tc.tile_pool(name="xcm", bufs=1) as xcmp,
            tc.tile_pool(name="w16", bufs=1) as w16p,
            tc.tile_pool(name="prod", bufs=1) as prodp,
            tc.tile_pool(name="outs", bufs=1) as outsp,
            tc.tile_pool(name="ps2", bufs=3, space="PSUM") as ps2p,
            tc.tile_pool(name="psr", bufs=4, space="PSUM") as psrp,
        ):
            ident = consts.tile([128, 128], bf16)
            make_identity(nc, ident[:])
            # preload the ACT copy table set early (one tiny copy)
            nc.scalar.copy(out=ident[0:1, 0:2], in_=ident[0:1, 0:2])
            w12a = consts.tile([128, M144], bf16)
            w12b = consts.tile([128, M144], bf16)
            
            nc.sync.dma_start(out=w12a[:], in_=w12_d[0])
            nc.sync.dma_start(out=w12b[:], in_=w12_d[1])
            brow = consts.tile([1, M144], bf16)
            nc.sync.dma_start(out=brow[:], in_=brow_d[:])
            ones1 = consts.tile([1, 128], bf16)
            nc.gpsimd.memset(ones1[:], 1.0)

            pswu = pswup.tile([128, 128], f32)
            for _ in range(30):
                nc.tensor.matmul(pswu[:], lhsT=ident[:], rhs=ident[:],
                                 start=True, stop=True)
            def dma_xcm_chunk(c):
                t = xcmp.tile([128, 2048], bf16, name=f"xcm_{c % 2}")
                nc.sync.dma_start(
                    out=t[:],
                    in_=AP(xcm_d, c * 2048, [[NCHUNK * 2048, 128], [1, 2048]]))
                return t

            # x views: one packed per-chunk tile (6 views x 2562), 2-deep
            XNAMES = ("x0e", "x0o", "xue", "xuo", "xde", "xdo")
            XIDX = {n: i for i, n in enumerate(XNAMES)}
            XCH = 10 * 256 + 2

            def dma_x_chunk(c):
                t = xbp.tile([128, 6 * XCH], bf16, name=f"xpk_{c % 2}")
                for h in range(2):
                    nc.sync.dma_start(
                        out=AP(t.tensor, h * 3 * XCH, [[6 * XCH, 128], [1, 3 * XCH]]),
                        in_=AP(xball_d, c * 6 * XCH + h * 3 * XCH,
                               [[NCHUNK * 6 * XCH, 128], [1, 3 * XCH]]),
                    )
                return t

            def t2gen(c, xcmt, w16t):
                # 8 tiles: wl = 8c+t ; ps2[pix,144] = xcm_tile.T @ W12 + bias
                for t in range(WLC):
                    ps2 = ps2p.tile([128, M144], f32)
                    nc.tensor.matmul(
                        ps2[:],
                        lhsT=AP(xcmt.tensor, t * 128, [[2048, 128], [1, 128]]),
                        rhs=w12a[:], start=True, stop=False)
                    nc.tensor.matmul(
                        ps2[:],
                        lhsT=AP(xcmt.tensor, 1024 + t * 128, [[2048, 128], [1, 128]]),
                        rhs=w12b[:], start=False, stop=bias_zero)
                    if not bias_zero:
                        nc.tensor.matmul(ps2[:], lhsT=ones1[:], rhs=brow[:],
                                         start=False, stop=True)
                    # scatter t2[9g+j] -> W16[t*256 + 16g+j]
                    nc.scalar.copy(
                        out=AP(w16t.tensor, t * 256,
                               [[2048, 128], [16, 16], [1, 9]]),
                        in_=AP(ps2.tensor, 0, [[M144, 128], [9, 16], [1, 9]]),
                    )

            XNAME = {(-1, 0): "xde", (-1, 1): "xdo", (0, 0): "x0e",
                     (0, 1): "x0o", (1, 0): "xue", (1, 1): "xuo"}

            def _bases(g, ki, gcs, jlo, xpar):
                di, dj = ki // 3 - 1, ki % 3 - 1
                voff = XIDX[XNAME[(di, xpar)]] * XCH
                ch0 = 144 * g + 9 * gcs + jlo - 256 * ki + xpar + voff
                col = (16 * g + gcs) - (128 if g >= 8 else 0)
                return ((dj + 1) * 256 + ch0, 16 * g + jlo, col * 16 + jlo)

            # pair single-row instrs of matching (half, njr) via a 3rd AP dim
            MERGED = {False: [], True: []}
            for want_b in (False, True):
                sub = [i for i in INSTRS if (i[0] >= 8) == want_b]
                singles = {}
                for ins in sub:
                    if ins[4] == 1:
                        singles.setdefault(ins[6], []).append(ins)
                    else:
                        MERGED[want_b].append(("s", ins))
                for njr, lst in singles.items():
                    lst.sort(key=lambda i: _bases(i[0], i[1], i[2], i[5], i[7])[2])
                    while len(lst) >= 2:
                        i1, i2 = lst[0], lst[1]
                        b1 = _bases(i1[0], i1[1], i1[2], i1[5], i1[7])
                        b2 = _bases(i2[0], i2[1], i2[2], i2[5], i2[7])
                        if b2[0] > b1[0] and b2[1] > b1[1] and b2[2] > b1[2]:
                            MERGED[want_b].append(("p", i1, i2, b1, b2))
                            lst = lst[2:]
                        else:
                            MERGED[want_b].append(("s", lst.pop(0)))
                    for ins in lst:
                        MERGED[want_b].append(("s", ins))

            def mults(c, xt, w16t, Ma, Mb, want_b):
                    Mx = Mb if want_b else Ma
                    for item in MERGED[want_b]:
                        if item[0] == "s":
                            (g, ki, gcs, step, n, jlo, njr, xpar) = item[1]
                            xb0, wb0, ob0 = _bases(g, ki, gcs, jlo, xpar)
                            in_dims = [[6 * XCH, 128], [256, WLC]]
                            w_dims = [[2048, 128], [256, WLC]]
                            o_dims = [[PF // WLC * WLC, 128], [2048, WLC]]
                            if n > 1:
                                in_dims.append([9 * step, n])
                                w_dims.append([0, n])
                                o_dims.append([16 * step, n])
                            in_dims.append([1, njr])
                            w_dims.append([1, njr])
                            o_dims.append([1, njr])
                            nc.vector.tensor_mul(
                                AP(Mx.tensor, ob0, o_dims),
                                AP(xt.tensor, xb0, in_dims),
                                AP(w16t.tensor, wb0, w_dims),
                            )
                        else:
                            (_, i1, i2, b1, b2) = item
                            njr = i1[6]
                            nc.vector.tensor_mul(
                                AP(Mx.tensor, b1[2],
                                   [[PF, 128], [b2[2] - b1[2], 2],
                                    [2048, WLC], [1, njr]]),
                                AP(xt.tensor, b1[0],
                                   [[6 * XCH, 128], [b2[0] - b1[0], 2],
                                    [256, WLC], [1, njr]]),
                                AP(w16t.tensor, b1[1],
                                   [[2048, 128], [b2[1] - b1[1], 2],
                                    [256, WLC], [1, njr]]),
                            )

            def fold_pe(c, Mx, half, outc):
                # 2 groups of 512 (4 wl x 128 co); 9 slot-plane accum-MMs each
                for k in range(2):
                    psf = psfp.tile([128, 512], f32)
                    for s in range(9):
                        nc.tensor.matmul(
                            psf[:],
                            lhsT=ident[:],
                            rhs=AP(Mx.tensor, 4 * k * 2048 + s,
                                   [[PF, 128], [2048, 4], [16, 128]]),
                            start=(s == 0), stop=(s == 8))
                    # outc free layout: [wl(8), co(256)]; half b at col 128
                    nc.scalar.copy(
                        out=AP(outc.tensor, (4 * k) * 256 + half * 128,
                               [[2048, 128], [256, 4], [1, 128]]),
                        in_=AP(psf.tensor, 0, [[512, 128], [128, 4], [1, 128]]),
                    )

            def fold_dve(c, Mx, half, outc):
                # F1 in place: M[4..7] += M[0..3]
                nc.vector.tensor_add(
                    AP(Mx.tensor, 4, [[PF, 128], [2048, WLC], [16, 128], [1, 4]]),
                    AP(Mx.tensor, 0, [[PF, 128], [2048, WLC], [16, 128], [1, 4]]),
                    AP(Mx.tensor, 4, [[PF, 128], [2048, WLC], [16, 128], [1, 4]]),
                )
                # PE: psf = planes 4..8 per 512-group (4 wl x 128 co)
                for k in range(2):
                    psr = psrp.tile([128, 512], f32)
                    for si, sl in enumerate((4, 5, 6, 7, 8)):
                        nc.tensor.matmul(
                            psr[:], lhsT=ident[:],
                            rhs=AP(Mx.tensor, 4 * k * 2048 + sl,
                                   [[PF, 128], [2048, 4], [16, 128]]),
                            start=(si == 0), stop=(si == 4))
                    nc.scalar.copy(
                        out=AP(outc.tensor, (4 * k) * 256 + half * 128,
                               [[2048, 128], [256, 4], [1, 128]]),
                        in_=AP(psr.tensor, 0, [[512, 128], [128, 4], [1, 128]]),
                    )

            W16t = [w16p.tile([128, WLC * 256], bf16, name=f"w16_{i}") for i in range(4)]
            Ma = prodp.tile([128, PF], bf16)
            Mb = prodp.tile([128, PF], bf16)
            OUTt = [outsp.tile([128, WLC * 256], bf16, name=f"out_{i}") for i in range(2)]

            xcms = {0: dma_xcm_chunk(0)}
            xts = {0: dma_x_chunk(0), 1: dma_x_chunk(1)}
            t2gen(0, xcms.pop(0), W16t[0])
            xcms[1] = dma_xcm_chunk(1)
            for c in range(NCHUNK):
                xt = xts.pop(c)
                outc = OUTt[c % 2]
                mults(c, xt, W16t[c], Ma, Mb, False)
                fold_dve(c, Ma, 0, outc)
                nc.sync.dma_start(
                    out=AP(out_d, c * WLC * 256, [[32 * 256, 128], [256, WLC], [1, 128]]),
                    in_=AP(outc.tensor, 0, [[WLC * 256, 128], [256, WLC], [1, 128]]),
                )
                if c + 1 < NCHUNK:
                    t2gen(c + 1, xcms.pop(c + 1), W16t[c + 1])
                if c + 2 < NCHUNK:
                    xcms[c + 2] = dma_xcm_chunk(c + 2)
                mults(c, xt, W16t[c], Ma, Mb, True)
                fold_dve(c, Mb, 1, outc)
                if c + 2 < NCHUNK:
                    xts[c + 2] = dma_x_chunk(c + 2)
                nc.sync.dma_start(
                    out=AP(out_d, c * WLC * 256 + 128,
                           [[32 * 256, 128], [256, WLC], [1, 128]]),
                    in_=AP(outc.tensor, 128, [[WLC * 256, 128], [256, WLC], [1, 128]]),
                )
    nc.compile()
    return nc


def _get_program(bias_zero=False):
    key = ("nc", bias_zero)
    if key not in _cache:
        _cache[key] = _build_program(bias_zero)
    return _cache[key]


XCH2 = 10 * 256 + 2


def _host_prep(x, W1, b1, W2, b2):
    bf = ml_dtypes.bfloat16
    W12 = (W1.astype(np.float64) @ W2.astype(np.float64)).astype(np.float32)
    bfused = (b1.astype(np.float64) @ W2.astype(np.float64) + b2).astype(np.float32)
    w12_h = np.ascontiguousarray(W12.astype(bf).reshape(2, 128, M144))
    brow_h = np.ascontiguousarray(bfused.astype(bf).reshape(1, M144))

    in_maps = []
    for i in range(x.shape[0]):
        xi = x[i].astype(bf)  # [64, 64, 256]
        # pixel-major: [p=64*wq+h, slot, ch]; slot s <-> w = 32*wq + s - 1
        xpm4 = np.zeros((2, 64, SLOTS, 256), dtype=bf)
        xpm4[0, :, 1:34, :] = xi[:, 0:33, :]
        xpm4[1, :, 0:33, :] = xi[:, 31:64, :]
        xpm = np.zeros((128, XF), dtype=bf)
        xpm[:, :SLOTS * 256] = xpm4.reshape(128, SLOTS * 256)
        xu = np.zeros_like(xpm)   # row h+1
        xd = np.zeros_like(xpm)   # row h-1
        for wq in range(2):
            xu[64 * wq:64 * wq + 63] = xpm[64 * wq + 1:64 * wq + 64]
            xd[64 * wq + 1:64 * wq + 64] = xpm[64 * wq:64 * wq + 63]
        def odd(a):
            o = np.zeros_like(a)
            o[:, 1:] = a[:, :-1]
            return o
        # channel-major packed per chunk: [ch%128, c, ch//128, wl_t*128+p]
        xc = xi.reshape(64, 2, 32, 256).transpose(3, 2, 1, 0).reshape(2, 128, 32, 128)
        # xc[h, p, wl, pix]; want [p, chunk, h, t*128+pix]
        xcm = np.ascontiguousarray(
            xc.reshape(2, 128, 4, 8, 128).transpose(1, 2, 0, 3, 4).reshape(128, -1))
        views = [xpm, odd(xpm), xu, odd(xu), xd, odd(xd)]
        xball = np.zeros((128, NCHUNK, 6, XCH2), dtype=bf)
        for c in range(NCHUNK):
            for v, a in enumerate(views):
                xball[:, c, v, :] = a[:, WLC * c * 256:WLC * c * 256 + XCH2]
        in_maps.append({
            "xball": np.ascontiguousarray(xball.reshape(128, -1)),
            "xcm": xcm, "w12": w12_h, "brow": brow_h,
        })
    return in_maps


def kernel(x, W1, b1, W2, b2, trace=False):
    from concourse.bass_utils import run_bass_kernel_spmd

    bfused = np.asarray(b1, np.float64) @ np.asarray(W2, np.float64) + np.asarray(b2)
    nc = _get_program(bias_zero=not np.any(bfused))
    in_maps = _host_prep(np.asarray(x), W1, b1, W2, b2)
    res = run_bass_kernel_spmd(nc, in_maps, core_ids=list(range(NCORES)),
                               trace=trace)
    outs = []
    for i in range(NCORES):
        buf = np.asarray(res.results[i]["out"]).astype(np.float32)
        # [64*wq+h, wl*256+c] -> [h, w, c]
        o = buf.reshape(2, 64, 32, 256).transpose(1, 0, 2, 3).reshape(64, 64, 256)
        outs.append(o)
    out = np.stack(outs, axis=0)
    if trace:
        return out, res
    return out


# revision 12
# speedup vs baseline: 1.3380x; 1.0295x over previous
# Trainium2 Bass kernel for nn_Invo2D (involution-style dynamic conv).
#
# Math:
#   t2 = x @ (W1@W2) + (b1@W2 + b2)            [pix, 144]   (both 1x1 convs are linear)
#   P[pix, f] = 3x3 SAME patches, f = ki*256 + ch
#   out[pix, co] = sum_j t2[pix, 9*(co//16)+j] * P[pix, 9*co+j]
#
# Sharding: data-parallel over batch, 1 image per core (8 cores).
#
# Layout: partition p = 64*wq + h; per-partition free = 34 w-slots x 256 ch
# (slot s <-> w = 32*wq + s - 1, halo zeroed on host). Host uploads bf16:
#  - 6 pixel-major x views: {row-1, row, row+1} x {even, odd(+1 elem shift)}
#    so every multiply AP is 4B-aligned step-1 (2x DVE packing)
#  - channel-major xcm for the weight matmul
# Products go to 16-padded (co,slot) blocks; fold = 9 identity-matmul
# accumulates into PSUM f32 on the tensor engine (or DVE tree fallback).

import numpy as np
import ml_dtypes

H, W, C = 64, 64, 256
G, GC = 16, 16
M144, D = 144, 64
NCORES = 8
SLOTS = 34
XF = SLOTS * C + 2      # 8706 (2 slack for odd-shifted views)
WLC = 8                 # wl per chunk
NCHUNK = 4
PF = WLC * 2048         # per-half product free size (8 wl x 128 co x 16 slots)

FOLD_ON_PE = False

_cache = {}


def _rect_decomp(r0, r1):
    out = []
    gc0, j0 = divmod(r0, 9)
    if j0 != 0:
        end = min(r1, (gc0 + 1) * 9)
        out.append((gc0, 1, j0, end - r0))
        r0 = end
        if r0 == r1:
            return out
        gc0, j0 = divmod(r0, 9)
    nfull = (r1 - r0) // 9
    if nfull:
        out.append((gc0, nfull, 0, 9))
        r0 += nfull * 9
        gc0 += nfull
    if r0 < r1:
        out.append((gc0, 1, 0, r1 - r0))
    return out


def _build_instrs():
    """Parity-aligned multiply instructions.
    Each: (g, ki, gc0, gstep, ngc, jlo, njr, xpar)."""
    instrs = []
    for g in range(16):
        f_lo, f_hi = 144 * g, 144 * g + 144
        cuts = [f_lo] + [256 * k for k in range(1, 9) if f_lo < 256 * k < f_hi] + [f_hi]
        for a, b in zip(cuts, cuts[1:]):
            ki = a // 256
            for (gc0, ngc, j0, nj) in _rect_decomp(a - 144 * g, b - 144 * g):
                if ngc == 1:
                    halves = [(gc0, 1, 1)]
                else:
                    halves = [(gc0, 2, (ngc + 1) // 2), (gc0 + 1, 2, ngc // 2)]
                    halves = [h for h in halves if h[2] > 0]
                for (gcs, step, n) in halves:
                    ch0 = 144 * g + 9 * gcs + j0 - 256 * ki
                    # left-extend odd-j0 runs by one (duplicate of the j0-1
                    # product) ONLY when it stays within this tap's channels
                    ext = 1 if (j0 % 2 == 1 and ch0 >= 1) else 0
                    xpar = (ch0 - ext) % 2
                    instrs.append((g, ki, gcs, step, n, j0 - ext, nj + ext, xpar))
    return instrs


def _build_program(bias_zero=False):
    import concourse.bass as bass
    import concourse.tile as tile
    from concourse import bacc, mybir
    from concourse.masks import make_identity

    f32 = mybir.dt.float32
    bf16 = mybir.dt.bfloat16
    AP = bass.AP

    nc = bacc.Bacc(None, target_bir_lowering=False)
    xball_d = nc.dram_tensor("xball", [128, NCHUNK * 6 * 2562], bf16,
                             kind="ExternalInput")
    xcm_d = nc.dram_tensor("xcm", [128, NCHUNK * 2048], bf16, kind="ExternalInput")
    w12_d = nc.dram_tensor("w12", [2, 128, M144], bf16, kind="ExternalInput")
    brow_d = nc.dram_tensor("brow", [1, M144], bf16, kind="ExternalInput")
    out_d = nc.dram_tensor("out", [128, 32 * 256], bf16, kind="ExternalOutput")

    INSTRS = _build_instrs()

    with tile.TileContext(nc) as tc:
        with (
            tc.tile_pool(name="consts", bufs=1) as consts,
            tc.tile_pool(name="xb", bufs=1) as xbp,
            tc.tile_pool(name="xcm", bufs=1) as xcmp,
            tc.tile_pool(name="w16", bufs=1) as w16p,
            tc.tile_pool(name="prod", bufs=1) as prodp,
            tc.tile_pool(name="outs", bufs=1) as outsp,
            tc.tile_pool(name="ps2", bufs=3, space="PSUM") as ps2p,
            tc.tile_pool(name="psr", bufs=4, space="PSUM") as psrp,
        ):
            ident = consts.tile([128, 128], bf16)
            make_identity(nc, ident[:])
            # preload the ACT copy table set early (one tiny copy)
            nc.scalar.copy(out=ident[0:1, 0:2], in_=ident[0:1, 0:2])
            w12a = consts.tile([128, M144], bf16)
            w12b = consts.tile([128, M144], bf16)
            
            nc.sync.dma_start(out=w12a[:], in_=w12_d[0])
            nc.sync.dma_start(out=w12b[:], in_=w12_d[1])
            brow = consts.tile([1, M144], bf16)
            nc.sync.dma_start(out=brow[:], in_=brow_d[:])
            ones1 = consts.tile([1, 128], bf16)
            nc.gpsimd.memset(ones1[:], 1.0)

            pswu = pswup.tile([128, 128], f32)
            for _ in range(30):
                nc.tensor.matmul(pswu[:], lhsT=ident[:], rhs=ident[:],
                                 start=True, stop=True)
            def dma_xcm_chunk(c):
                t = xcmp.tile([128, 2048], bf16, name=f"xcm_{c % 2}")
                nc.sync.dma_start(
                    out=t[:],
                    in_=AP(xcm_d, c * 2048, [[NCHUNK * 2048, 128], [1, 2048]]))
                return t

            # x views: one packed per-chunk tile (6 views x 2562), 2-deep
            XNAMES = ("x0e", "x0o", "xue", "xuo", "xde", "xdo")
            XIDX = {n: i for i, n in enumerate(XNAMES)}
            XCH = 10 * 256 + 2

            def dma_x_chunk(c):
                t = xbp.tile([128, 6 * XCH], bf16, name=f"xpk_{c % 2}")
                for h in range(2):
                    nc.sync.dma_start(
                        out=AP(t.tensor, h * 3 * XCH, [[6 * XCH, 128], [1, 3 * XCH]]),
                        in_=AP(xball_d, c * 6 * XCH + h * 3 * XCH,
                               [[NCHUNK * 6 * XCH, 128], [1, 3 * XCH]]),
                    )
                return t

            def t2gen(c, xcmt, w16t):
                # 8 tiles: wl = 8c+t ; ps2[pix,144] = xcm_tile.T @ W12 + bias
                for t in range(WLC):
                    ps2 = ps2p.tile([128, M144], f32)
                    nc.tensor.matmul(
                        ps2[:],
                        lhsT=AP(xcmt.tensor, t * 128, [[2048, 128], [1, 128]]),
                        rhs=w12a[:], start=True, stop=False)
                    nc.tensor.matmul(
                        ps2[:],
                        lhsT=AP(xcmt.tensor, 1024 + t * 128, [[2048, 128], [1, 128]]),
                        rhs=w12b[:], start=False, stop=bias_zero)
                    if not bias_zero:
                        nc.tensor.matmul(ps2[:], lhsT=ones1[:], rhs=brow[:],
                                         start=False, stop=True)
                    # scatter t2[9g+j] -> W16[t*256 + 16g+j]
                    nc.scalar.copy(
                        out=AP(w16t.tensor, t * 256,
                               [[2048, 128], [16, 16], [1, 9]]),
                        in_=AP(ps2.tensor, 0, [[M144, 128], [9, 16], [1, 9]]),
                    )

            XNAME = {(-1, 0): "xde", (-1, 1): "xdo", (0, 0): "x0e",
                     (0, 1): "x0o", (1, 0): "xue", (1, 1): "xuo"}

            def _bases(g, ki, gcs, jlo, xpar):
                di, dj = ki // 3 - 1, ki % 3 - 1
                voff = XIDX[XNAME[(di, xpar)]] * XCH
                ch0 = 144 * g + 9 * gcs + jlo - 256 * ki + xpar + voff
                col = (16 * g + gcs) - (128 if g >= 8 else 0)
                return ((dj + 1) * 256 + ch0, 16 * g + jlo, col * 16 + jlo)

            # pair single-row instrs of matching (half, njr) via a 3rd AP dim
            MERGED = {False: [], True: []}
            for want_b in (False, True):
                sub = [i for i in INSTRS if (i[0] >= 8) == want_b]
                singles = {}
                for ins in sub:
                    if ins[4] == 1:
                        singles.setdefault(ins[6], []).append(ins)
                    else:
                        MERGED[want_b].append(("s", ins))
                for njr, lst in singles.items():
                    lst.sort(key=lambda i: _bases(i[0], i[1], i[2], i[5], i[7])[2])
                    while len(lst) >= 2:
                        i1, i2 = lst[0], lst[1]
                        b1 = _bases(i1[0], i1[1], i1[2], i1[5], i1[7])
                        b2 = _bases(i2[0], i2[1], i2[2], i2[5], i2[7])
                        if b2[0] > b1[0] and b2[1] > b1[1] and b2[2] > b1[2]:
                            MERGED[want_b].append(("p", i1, i2, b1, b2))
                            lst = lst[2:]
                        else:
                            MERGED[want_b].append(("s", lst.pop(0)))
                    for ins in lst:
                        MERGED[want_b].append(("s", ins))

            def mults(c, xt, w16t, Ma, Mb, want_b):
                    Mx = Mb if want_b else Ma
                    for item in MERGED[want_b]:
                        if item[0] == "s":
                            (g, ki, gcs, step, n, jlo, njr, xpar) = item[1]
                            xb0, wb0, ob0 = _bases(g, ki, gcs, jlo, xpar)
                            in_dims = [[6 * XCH, 128], [256, WLC]]
                            w_dims = [[2048, 128], [256, WLC]]
                            o_dims = [[PF // WLC * WLC, 128], [2048, WLC]]
                            if n > 1:
                                in_dims.append([9 * step, n])
                                w_dims.append([0, n])
                                o_dims.append([16 * step, n])
                            in_dims.append([1, njr])
                            w_dims.append([1, njr])
                            o_dims.append([1, njr])
                            nc.vector.tensor_mul(
                                AP(Mx.tensor, ob0, o_dims),
                                AP(xt.tensor, xb0, in_dims),
                                AP(w16t.tensor, wb0, w_dims),
                            )
                        else:
                            (_, i1, i2, b1, b2) = item
                            njr = i1[6]
                            nc.vector.tensor_mul(
                                AP(Mx.tensor, b1[2],
                                   [[PF, 128], [b2[2] - b1[2], 2],
                                    [2048, WLC], [1, njr]]),
                                AP(xt.tensor, b1[0],
                                   [[6 * XCH, 128], [b2[0] - b1[0], 2],
                                    [256, WLC], [1, njr]]),
                                AP(w16t.tensor, b1[1],
                                   [[2048, 128], [b2[1] - b1[1], 2],
                                    [256, WLC], [1, njr]]),
                            )

            def fold_pe(c, Mx, half, outc):
                # 2 groups of 512 (4 wl x 128 co); 9 slot-plane accum-MMs each
                for k in range(2):
                    psf = psfp.tile([128, 512], f32)
                    for s in range(9):
                        nc.tensor.matmul(
                            psf[:],
                            lhsT=ident[:],
                            rhs=AP(Mx.tensor, 4 * k * 2048 + s,
                                   [[PF, 128], [2048, 4], [16, 128]]),
                            start=(s == 0), stop=(s == 8))
                    # outc free layout: [wl(8), co(256)]; half b at col 128
                    nc.scalar.copy(
                        out=AP(outc.tensor, (4 * k) * 256 + half * 128,
                               [[2048, 128], [256, 4], [1, 128]]),
                        in_=AP(psf.tensor, 0, [[512, 128], [128, 4], [1, 128]]),
                    )

            def fold_dve(c, Mx, half, outc):
                # F1 in place: M[4..7] += M[0..3]
                nc.vector.tensor_add(
                    AP(Mx.tensor, 4, [[PF, 128], [2048, WLC], [16, 128], [1, 4]]),
                    AP(Mx.tensor, 0, [[PF, 128], [2048, WLC], [16, 128], [1, 4]]),
                    AP(Mx.tensor, 4, [[PF, 128], [2048, WLC], [16, 128], [1, 4]]),
                )
                # PE: psf = planes 4..8 per 512-group (4 wl x 128 co)
                for k in range(2):
                    psr = psrp.tile([128, 512], f32)
                    for si, sl in enumerate((4, 5, 6, 7, 8)):
                        nc.tensor.matmul(
                            psr[:], lhsT=ident[:],
                            rhs=AP(Mx.tensor, 4 * k * 2048 + sl,
                                   [[PF, 128], [2048, 4], [16, 128]]),
                            start=(si == 0), stop=(si == 4))
                    nc.scalar.copy(
                        out=AP(outc.tensor, (4 * k) * 256 + half * 128,
                               [[2048, 128], [256, 4], [1, 128]]),
                        in_=AP(psr.tensor, 0, [[512, 128], [128, 4], [1, 128]]),
                    )

            W16t = [w16p.tile([128, WLC * 256], bf16, name=f"w16_{i}") for i in range(4)]
            Ma = prodp.tile([128, PF], bf16)
            Mb = prodp.tile([128, PF], bf16)
            OUTt = [outsp.tile([128, WLC * 256], bf16, name=f"out_{i}") for i in range(2)]

            xcms = {0: dma_xcm_chunk(0)}
            xcms[1] = dma_xcm_chunk(1)
            xts = {0: dma_x_chunk(0), 1: dma_x_chunk(1)}
            t2gen(0, xcms.pop(0), W16t[0])
            for c in range(NCHUNK):
                xt = xts.pop(c)
                outc = OUTt[c % 2]
                mults(c, xt, W16t[c], Ma, Mb, False)
                fold_dve(c, Ma, 0, outc)
                nc.sync.dma_start(
                    out=AP(out_d, c * WLC * 256, [[32 * 256, 128], [256, WLC], [1, 128]]),
                    in_=AP(outc.tensor, 0, [[WLC * 256, 128], [256, WLC], [1, 128]]),
                )
                if c + 1 < NCHUNK:
                    t2gen(c + 1, xcms.pop(c + 1), W16t[c + 1])
                if c + 2 < NCHUNK:
                    xcms[c + 2] = dma_xcm_chunk(c + 2)
                mults(c, xt, W16t[c], Ma, Mb, True)
                fold_dve(c, Mb, 1, outc)
                if c + 2 < NCHUNK:
                    xts[c + 2] = dma_x_chunk(c + 2)
                nc.sync.dma_start(
                    out=AP(out_d, c * WLC * 256 + 128,
                           [[32 * 256, 128], [256, WLC], [1, 128]]),
                    in_=AP(outc.tensor, 128, [[WLC * 256, 128], [256, WLC], [1, 128]]),
                )
    nc.compile()
    return nc


def _get_program(bias_zero=False):
    key = ("nc", bias_zero)
    if key not in _cache:
        _cache[key] = _build_program(bias_zero)
    return _cache[key]


XCH2 = 10 * 256 + 2


def _host_prep(x, W1, b1, W2, b2):
    bf = ml_dtypes.bfloat16
    W12 = (W1.astype(np.float64) @ W2.astype(np.float64)).astype(np.float32)
    bfused = (b1.astype(np.float64) @ W2.astype(np.float64) + b2).astype(np.float32)
    w12_h = np.ascontiguousarray(W12.astype(bf).reshape(2, 128, M144))
    brow_h = np.ascontiguousarray(bfused.astype(bf).reshape(1, M144))

    in_maps = []
    for i in range(x.shape[0]):
        xi = x[i].astype(bf)  # [64, 64, 256]
        # pixel-major: [p=64*wq+h, slot, ch]; slot s <-> w = 32*wq + s - 1
        xpm4 = np.zeros((2, 64, SLOTS, 256), dtype=bf)
        xpm4[0, :, 1:34, :] = xi[:, 0:33, :]
        xpm4[1, :, 0:33, :] = xi[:, 31:64, :]
        xpm = np.zeros((128, XF), dtype=bf)
        xpm[:, :SLOTS * 256] = xpm4.reshape(128, SLOTS * 256)
        xu = np.zeros_like(xpm)   # row h+1
        xd = np.zeros_like(xpm)   # row h-1
        for wq in range(2):
            xu[64 * wq:64 * wq + 63] = xpm[64 * wq + 1:64 * wq + 64]
            xd[64 * wq + 1:64 * wq + 64] = xpm[64 * wq:64 * wq + 63]
        def odd(a):
            o = np.zeros_like(a)
            o[:, 1:] = a[:, :-1]
            return o
        # channel-major packed per chunk: [ch%128, c, ch//128, wl_t*128+p]
        xc = xi.reshape(64, 2, 32, 256).transpose(3, 2, 1, 0).reshape(2, 128, 32, 128)
        # xc[h, p, wl, pix]; want [p, chunk, h, t*128+pix]
        xcm = np.ascontiguousarray(
            xc.reshape(2, 128, 4, 8, 128).transpose(1, 2, 0, 3, 4).reshape(128, -1))
        views = [xpm, odd(xpm), xu, odd(xu), xd, odd(xd)]
        xball = np.zeros((128, NCHUNK, 6, XCH2), dtype=bf)
        for c in range(NCHUNK):
            for v, a in enumerate(views):
                xball[:, c, v, :] = a[:, WLC * c * 256:WLC * c * 256 + XCH2]
        in_maps.append({
            "xball": np.ascontiguousarray(xball.reshape(128, -1)),
            "xcm": xcm, "w12": w12_h, "brow": brow_h,
        })
    return in_maps


def kernel(x, W1, b1, W2, b2, trace=False):
    from concourse.bass_utils import run_bass_kernel_spmd

    bfused = np.asarray(b1, np.float64) @ np.asarray(W2, np.float64) + np.asarray(b2)
    nc = _get_program(bias_zero=not np.any(bfused))
    in_maps = _host_prep(np.asarray(x), W1, b1, W2, b2)
    res = run_bass_kernel_spmd(nc, in_maps, core_ids=list(range(NCORES)),
                               trace=trace)
    outs = []
    for i in range(NCORES):
        buf = np.asarray(res.results[i]["out"]).astype(np.float32)
        # [64*wq+h, wl*256+c] -> [h, w, c]
        o = buf.reshape(2, 64, 32, 256).transpose(1, 0, 2, 3).reshape(64, 64, 256)
        outs.append(o)
    out = np.stack(outs, axis=0)
    if trace:
        return out, res
    return out


# revision 13
# speedup vs baseline: 1.3529x; 1.0112x over previous
# Trainium2 Bass kernel for nn_Invo2D (involution-style dynamic conv).
#
# Math:
#   t2 = x @ (W1@W2) + (b1@W2 + b2)            [pix, 144]   (both 1x1 convs are linear)
#   P[pix, f] = 3x3 SAME patches, f = ki*256 + ch
#   out[pix, co] = sum_j t2[pix, 9*(co//16)+j] * P[pix, 9*co+j]
#
# Sharding: data-parallel over batch, 1 image per core (8 cores).
#
# Layout: partition p = 64*wq + h; per-partition free = 34 w-slots x 256 ch
# (slot s <-> w = 32*wq + s - 1, halo zeroed on host). Host uploads bf16:
#  - 6 pixel-major x views: {row-1, row, row+1} x {even, odd(+1 elem shift)}
#    so every multiply AP is 4B-aligned step-1 (2x DVE packing)
#  - channel-major xcm for the weight matmul
# Products go to 16-padded (co,slot) blocks; fold = 9 identity-matmul
# accumulates into PSUM f32 on the tensor engine (or DVE tree fallback).

import numpy as np
import ml_dtypes

H, W, C = 64, 64, 256
G, GC = 16, 16
M144, D = 144, 64
NCORES = 8
SLOTS = 34
XF = SLOTS * C + 2      # 8706 (2 slack for odd-shifted views)
WLC = 8                 # wl per chunk
NCHUNK = 4
PF = WLC * 2048         # per-half product free size (8 wl x 128 co x 16 slots)

FOLD_ON_PE = False

_cache = {}


def _rect_decomp(r0, r1):
    out = []
    gc0, j0 = divmod(r0, 9)
    if j0 != 0:
        end = min(r1, (gc0 + 1) * 9)
        out.append((gc0, 1, j0, end - r0))
        r0 = end
        if r0 == r1:
            return out
        gc0, j0 = divmod(r0, 9)
    nfull = (r1 - r0) // 9
    if nfull:
        out.append((gc0, nfull, 0, 9))
        r0 += nfull * 9
        gc0 += nfull
    if r0 < r1:
        out.append((gc0, 1, 0, r1 - r0))
    return out


def _build_instrs():
    """Parity-aligned multiply instructions.
    Each: (g, ki, gc0, gstep, ngc, jlo, njr, xpar)."""
    instrs = []
    for g in range(16):
        f_lo, f_hi = 144 * g, 144 * g + 144
        cuts = [f_lo] + [256 * k for k in range(1, 9) if f_lo < 256 * k < f_hi] + [f_hi]
        for a, b in zip(cuts, cuts[1:]):
            ki = a // 256
            for (gc0, ngc, j0, nj) in _rect_decomp(a - 144 * g, b - 144 * g):
                if ngc == 1:
                    halves = [(gc0, 1, 1)]
                else:
                    halves = [(gc0, 2, (ngc + 1) // 2), (gc0 + 1, 2, ngc // 2)]
                    halves = [h for h in halves if h[2] > 0]
                for (gcs, step, n) in halves:
                    ch0 = 144 * g + 9 * gcs + j0 - 256 * ki
                    # left-extend odd-j0 runs by one (duplicate of the j0-1
                    # product) ONLY when it stays within this tap's channels
                    ext = 1 if (j0 % 2 == 1 and ch0 >= 1) else 0
                    xpar = (ch0 - ext) % 2
                    instrs.append((g, ki, gcs, step, n, j0 - ext, nj + ext, xpar))
    return instrs


def _build_program(bias_zero=False):
    import concourse.bass as bass
    import concourse.tile as tile
    from concourse import bacc, mybir
    from concourse.masks import make_identity

    f32 = mybir.dt.float32
    bf16 = mybir.dt.bfloat16
    AP = bass.AP

    nc = bacc.Bacc(None, target_bir_lowering=False)
    xball_d = nc.dram_tensor("xball", [128, NCHUNK * 6 * 2562], bf16,
                             kind="ExternalInput")
    xcm_d = nc.dram_tensor("xcm", [128, NCHUNK * 2048], bf16, kind="ExternalInput")
    w12_d = nc.dram_tensor("w12", [2, 128, M144], bf16, kind="ExternalInput")
    brow_d = nc.dram_tensor("brow", [1, M144], bf16, kind="ExternalInput")
    out_d = nc.dram_tensor("out", [128, 32 * 256], bf16, kind="ExternalOutput")

    INSTRS = _build_instrs()

    with tile.TileContext(nc) as tc:
        with (
            tc.tile_pool(name="consts", bufs=1) as consts,
            tc.tile_pool(name="xb", bufs=1) as xbp,
            tc.tile_pool(name="xcm", bufs=1) as xcmp,
            tc.tile_pool(name="w16", bufs=1) as w16p,
            tc.tile_pool(name="prod", bufs=1) as prodp,
            tc.tile_pool(name="outs", bufs=1) as outsp,
            tc.tile_pool(name="ps2", bufs=3, space="PSUM") as ps2p,
            tc.tile_pool(name="psr", bufs=4, space="PSUM") as psrp,
        ):
            ident = consts.tile([128, 128], bf16)
            make_identity(nc, ident[:])
            # preload the ACT copy table set early (one tiny copy)
            nc.scalar.copy(out=ident[0:1, 0:2], in_=ident[0:1, 0:2])
            w12a = consts.tile([128, M144], bf16)
            w12b = consts.tile([128, M144], bf16)
            
            nc.sync.dma_start(out=w12a[:], in_=w12_d[0])
            nc.sync.dma_start(out=w12b[:], in_=w12_d[1])
            brow = consts.tile([1, M144], bf16)
            nc.sync.dma_start(out=brow[:], in_=brow_d[:])
            ones1 = consts.tile([1, 128], bf16)
            nc.gpsimd.memset(ones1[:], 1.0)

            pswu = pswup.tile([128, 128], f32)
            for _ in range(30):
                nc.tensor.matmul(pswu[:], lhsT=ident[:], rhs=ident[:],
                                 start=True, stop=True)
            def dma_xcm_chunk(c):
                t = xcmp.tile([128, 2048], bf16, name=f"xcm_{c % 2}")
                nc.sync.dma_start(
                    out=t[:],
                    in_=AP(xcm_d, c * 2048, [[NCHUNK * 2048, 128], [1, 2048]]))
                return t

            # x views: one packed per-chunk tile (6 views x 2562), 2-deep
            XNAMES = ("x0e", "x0o", "xue", "xuo", "xde", "xdo")
            XIDX = {n: i for i, n in enumerate(XNAMES)}
            XCH = 10 * 256 + 2

            def dma_x_chunk(c):
                t = xbp.tile([128, 6 * XCH], bf16, name=f"xpk_{c % 2}")
                for h in range(3):
                    nc.sync.dma_start(
                        out=AP(t.tensor, h * 2 * XCH, [[6 * XCH, 128], [1, 2 * XCH]]),
                        in_=AP(xball_d, c * 6 * XCH + h * 2 * XCH,
                               [[NCHUNK * 6 * XCH, 128], [1, 2 * XCH]]),
                    )
                return t

            def t2gen(c, xcmt, w16t):
                # 8 tiles: wl = 8c+t ; ps2[pix,144] = xcm_tile.T @ W12 + bias
                for t in range(WLC):
                    ps2 = ps2p.tile([128, M144], f32)
                    nc.tensor.matmul(
                        ps2[:],
                        lhsT=AP(xcmt.tensor, t * 128, [[2048, 128], [1, 128]]),
                        rhs=w12a[:], start=True, stop=False)
                    nc.tensor.matmul(
                        ps2[:],
                        lhsT=AP(xcmt.tensor, 1024 + t * 128, [[2048, 128], [1, 128]]),
                        rhs=w12b[:], start=False, stop=bias_zero)
                    if not bias_zero:
                        nc.tensor.matmul(ps2[:], lhsT=ones1[:], rhs=brow[:],
                                         start=False, stop=True)
                    # scatter t2[9g+j] -> W16[t*256 + 16g+j]
                    nc.scalar.copy(
                        out=AP(w16t.tensor, t * 256,
                               [[2048, 128], [16, 16], [1, 9]]),
                        in_=AP(ps2.tensor, 0, [[M144, 128], [9, 16], [1, 9]]),
                    )

            XNAME = {(-1, 0): "xde", (-1, 1): "xdo", (0, 0): "x0e",
                     (0, 1): "x0o", (1, 0): "xue", (1, 1): "xuo"}

            def _bases(g, ki, gcs, jlo, xpar):
                di, dj = ki // 3 - 1, ki % 3 - 1
                voff = XIDX[XNAME[(di, xpar)]] * XCH
                ch0 = 144 * g + 9 * gcs + jlo - 256 * ki + xpar + voff
                col = (16 * g + gcs) - (128 if g >= 8 else 0)
                return ((dj + 1) * 256 + ch0, 16 * g + jlo, col * 16 + jlo)

            # pair single-row instrs of matching (half, njr) via a 3rd AP dim
            MERGED = {False: [], True: []}
            for want_b in (False, True):
                sub = [i for i in INSTRS if (i[0] >= 8) == want_b]
                singles = {}
                for ins in sub:
                    if ins[4] == 1:
                        singles.setdefault(ins[6], []).append(ins)
                    else:
                        MERGED[want_b].append(("s", ins))
                for njr, lst in singles.items():
                    lst.sort(key=lambda i: _bases(i[0], i[1], i[2], i[5], i[7])[2])
                    while len(lst) >= 2:
                        i1, i2 = lst[0], lst[1]
                        b1 = _bases(i1[0], i1[1], i1[2], i1[5], i1[7])
                        b2 = _bases(i2[0], i2[1], i2[2], i2[5], i2[7])
                        if b2[0] > b1[0] and b2[1] > b1[1] and b2[2] > b1[2]:
                            MERGED[want_b].append(("p", i1, i2, b1, b2))
                            lst = lst[2:]
                        else:
                            MERGED[want_b].append(("s", lst.pop(0)))
                    for ins in lst:
                        MERGED[want_b].append(("s", ins))

            def mults(c, xt, w16t, Ma, Mb, want_b):
                    Mx = Mb if want_b else Ma
                    for item in MERGED[want_b]:
                        if item[0] == "s":
                            (g, ki, gcs, step, n, jlo, njr, xpar) = item[1]
                            xb0, wb0, ob0 = _bases(g, ki, gcs, jlo, xpar)
                            in_dims = [[6 * XCH, 128], [256, WLC]]
                            w_dims = [[2048, 128], [256, WLC]]
                            o_dims = [[PF // WLC * WLC, 128], [2048, WLC]]
                            if n > 1:
                                in_dims.append([9 * step, n])
                                w_dims.append([0, n])
                                o_dims.append([16 * step, n])
                            in_dims.append([1, njr])
                            w_dims.append([1, njr])
                            o_dims.append([1, njr])
                            nc.vector.tensor_mul(
                                AP(Mx.tensor, ob0, o_dims),
                                AP(xt.tensor, xb0, in_dims),
                                AP(w16t.tensor, wb0, w_dims),
                            )
                        else:
                            (_, i1, i2, b1, b2) = item
                            njr = i1[6]
                            nc.vector.tensor_mul(
                                AP(Mx.tensor, b1[2],
                                   [[PF, 128], [b2[2] - b1[2], 2],
                                    [2048, WLC], [1, njr]]),
                                AP(xt.tensor, b1[0],
                                   [[6 * XCH, 128], [b2[0] - b1[0], 2],
                                    [256, WLC], [1, njr]]),
                                AP(w16t.tensor, b1[1],
                                   [[2048, 128], [b2[1] - b1[1], 2],
                                    [256, WLC], [1, njr]]),
                            )

            def fold_pe(c, Mx, half, outc):
                # 2 groups of 512 (4 wl x 128 co); 9 slot-plane accum-MMs each
                for k in range(2):
                    psf = psfp.tile([128, 512], f32)
                    for s in range(9):
                        nc.tensor.matmul(
                            psf[:],
                            lhsT=ident[:],
                            rhs=AP(Mx.tensor, 4 * k * 2048 + s,
                                   [[PF, 128], [2048, 4], [16, 128]]),
                            start=(s == 0), stop=(s == 8))
                    # outc free layout: [wl(8), co(256)]; half b at col 128
                    nc.scalar.copy(
                        out=AP(outc.tensor, (4 * k) * 256 + half * 128,
                               [[2048, 128], [256, 4], [1, 128]]),
                        in_=AP(psf.tensor, 0, [[512, 128], [128, 4], [1, 128]]),
                    )

            def fold_dve(c, Mx, half, outc):
                # F1 in place: M[4..7] += M[0..3]
                nc.vector.tensor_add(
                    AP(Mx.tensor, 4, [[PF, 128], [2048, WLC], [16, 128], [1, 4]]),
                    AP(Mx.tensor, 0, [[PF, 128], [2048, WLC], [16, 128], [1, 4]]),
                    AP(Mx.tensor, 4, [[PF, 128], [2048, WLC], [16, 128], [1, 4]]),
                )
                # PE: psf = planes 4..8 per 512-group (4 wl x 128 co)
                for k in range(2):
                    psr = psrp.tile([128, 512], f32)
                    for si, sl in enumerate((4, 5, 6, 7, 8)):
                        nc.tensor.matmul(
                            psr[:], lhsT=ident[:],
                            rhs=AP(Mx.tensor, 4 * k * 2048 + sl,
                                   [[PF, 128], [2048, 4], [16, 128]]),
                            start=(si == 0), stop=(si == 4))
                    nc.scalar.copy(
                        out=AP(outc.tensor, (4 * k) * 256 + half * 128,
                               [[2048, 128], [256, 4], [1, 128]]),
                        in_=AP(psr.tensor, 0, [[512, 128], [128, 4], [1, 128]]),
                    )

            W16t = [w16p.tile([128, WLC * 256], bf16, name=f"w16_{i}") for i in range(4)]
            Ma = prodp.tile([128, PF], bf16)
            Mb = prodp.tile([128, PF], bf16)
            OUTt = [outsp.tile([128, WLC * 256], bf16, name=f"out_{i}") for i in range(2)]

            xcms = {0: dma_xcm_chunk(0)}
            xcms[1] = dma_xcm_chunk(1)
            xts = {0: dma_x_chunk(0), 1: dma_x_chunk(1)}
            t2gen(0, xcms.pop(0), W16t[0])
            for c in range(NCHUNK):
                xt = xts.pop(c)
                outc = OUTt[c % 2]
                mults(c, xt, W16t[c], Ma, Mb, False)
                fold_dve(c, Ma, 0, outc)
                nc.sync.dma_start(
                    out=AP(out_d, c * WLC * 256, [[32 * 256, 128], [256, WLC], [1, 128]]),
                    in_=AP(outc.tensor, 0, [[WLC * 256, 128], [256, WLC], [1, 128]]),
                )
                if c + 1 < NCHUNK:
                    t2gen(c + 1, xcms.pop(c + 1), W16t[c + 1])
                if c + 2 < NCHUNK:
                    xcms[c + 2] = dma_xcm_chunk(c + 2)
                mults(c, xt, W16t[c], Ma, Mb, True)
                fold_dve(c, Mb, 1, outc)
                if c + 2 < NCHUNK:
                    xts[c + 2] = dma_x_chunk(c + 2)
                nc.sync.dma_start(
                    out=AP(out_d, c * WLC * 256 + 128,
                           [[32 * 256, 128], [256, WLC], [1, 128]]),
                    in_=AP(outc.tensor, 128, [[WLC * 256, 128], [256, WLC], [1, 128]]),
                )
    nc.compile()
    return nc


def _get_program(bias_zero=False):
    key = ("nc", bias_zero)
    if key not in _cache:
        _cache[key] = _build_program(bias_zero)
    return _cache[key]


XCH2 = 10 * 256 + 2


def _host_prep(x, W1, b1, W2, b2):
    bf = ml_dtypes.bfloat16
    W12 = (W1.astype(np.float64) @ W2.astype(np.float64)).astype(np.float32)
    bfused = (b1.astype(np.float64) @ W2.astype(np.float64) + b2).astype(np.float32)
    w12_h = np.ascontiguousarray(W12.astype(bf).reshape(2, 128, M144))
    brow_h = np.ascontiguousarray(bfused.astype(bf).reshape(1, M144))

    in_maps = []
    for i in range(x.shape[0]):
        xi = x[i].astype(bf)  # [64, 64, 256]
        # pixel-major: [p=64*wq+h, slot, ch]; slot s <-> w = 32*wq + s - 1
        xpm4 = np.zeros((2, 64, SLOTS, 256), dtype=bf)
        xpm4[0, :, 1:34, :] = xi[:, 0:33, :]
        xpm4[1, :, 0:33, :] = xi[:, 31:64, :]
        xpm = np.zeros((128, XF), dtype=bf)
        xpm[:, :SLOTS * 256] = xpm4.reshape(128, SLOTS * 256)
        xu = np.zeros_like(xpm)   # row h+1
        xd = np.zeros_like(xpm)   # row h-1
        for wq in range(2):
            xu[64 * wq:64 * wq + 63] = xpm[64 * wq + 1:64 * wq + 64]
            xd[64 * wq + 1:64 * wq + 64] = xpm[64 * wq:64 * wq + 63]
        def odd(a):
            o = np.zeros_like(a)
            o[:, 1:] = a[:, :-1]
            return o
        # channel-major packed per chunk: [ch%128, c, ch//128, wl_t*128+p]
        xc = xi.reshape(64, 2, 32, 256).transpose(3, 2, 1, 0).reshape(2, 128, 32, 128)
        # xc[h, p, wl, pix]; want [p, chunk, h, t*128+pix]
        xcm = np.ascontiguousarray(
            xc.reshape(2, 128, 4, 8, 128).transpose(1, 2, 0, 3, 4).reshape(128, -1))
        views = [xpm, odd(xpm), xu, odd(xu), xd, odd(xd)]
        xball = np.zeros((128, NCHUNK, 6, XCH2), dtype=bf)
        for c in range(NCHUNK):
            for v, a in enumerate(views):
                xball[:, c, v, :] = a[:, WLC * c * 256:WLC * c * 256 + XCH2]
        in_maps.append({
            "xball": np.ascontiguousarray(xball.reshape(128, -1)),
            "xcm": xcm, "w12": w12_h, "brow": brow_h,
        })
    return in_maps


def kernel(x, W1, b1, W2, b2, trace=False):
    from concourse.bass_utils import run_bass_kernel_spmd

    bfused = np.asarray(b1, np.float64) @ np.asarray(W2, np.float64) + np.asarray(b2)
    nc = _get_program(bias_zero=not np.any(bfused))
    in_maps = _host_prep(np.asarray(x), W1, b1, W2, b2)
    res = run_bass_kernel_spmd(nc, in_maps, core_ids=list(range(NCORES)),
                               trace=trace)
    outs = []
    for i in range(NCORES):
        buf = np.asarray(res.results[i]["out"]).astype(np.float32)
        # [64*wq+h, wl*256+c] -> [h, w, c]
        o = buf.reshape(2, 64, 32, 256).transpose(1, 0, 2, 3).reshape(64, 64, 256)
        outs.append(o)
    out = np.stack(outs, axis=0)
    if trace:
        return out, res
    return out


# revision 14
# speedup vs baseline: 1.3644x; 1.0085x over previous
# Trainium2 Bass kernel for nn_Invo2D (involution-style dynamic conv).
#
# Math:
#   t2 = x @ (W1@W2) + (b1@W2 + b2)            [pix, 144]   (both 1x1 convs are linear)
#   P[pix, f] = 3x3 SAME patches, f = ki*256 + ch
#   out[pix, co] = sum_j t2[pix, 9*(co//16)+j] * P[pix, 9*co+j]
#
# Sharding: data-parallel over batch, 1 image per core (8 cores).
#
# Layout: partition p = 64*wq + h; per-partition free = 34 w-slots x 256 ch
# (slot s <-> w = 32*wq + s - 1, halo zeroed on host). Host uploads bf16:
#  - 6 pixel-major x views: {row-1, row, row+1} x {even, odd(+1 elem shift)}
#    so every multiply AP is 4B-aligned step-1 (2x DVE packing)
#  - channel-major xcm for the weight matmul
# Products go to 16-padded (co,slot) blocks; fold = 9 identity-matmul
# accumulates into PSUM f32 on the tensor engine (or DVE tree fallback).

import numpy as np
import ml_dtypes

H, W, C = 64, 64, 256
G, GC = 16, 16
M144, D = 144, 64
NCORES = 8
SLOTS = 34
XF = SLOTS * C + 2      # 8706 (2 slack for odd-shifted views)
WLC = 8                 # wl per chunk
NCHUNK = 4
PF = WLC * 2048         # per-half product free size (8 wl x 128 co x 16 slots)

FOLD_ON_PE = False

_cache = {}


def _rect_decomp(r0, r1):
    out = []
    gc0, j0 = divmod(r0, 9)
    if j0 != 0:
        end = min(r1, (gc0 + 1) * 9)
        out.append((gc0, 1, j0, end - r0))
        r0 = end
        if r0 == r1:
            return out
        gc0, j0 = divmod(r0, 9)
    nfull = (r1 - r0) // 9
    if nfull:
        out.append((gc0, nfull, 0, 9))
        r0 += nfull * 9
        gc0 += nfull
    if r0 < r1:
        out.append((gc0, 1, 0, r1 - r0))
    return out


def _build_instrs():
    """Parity-aligned multiply instructions.
    Each: (g, ki, gc0, gstep, ngc, jlo, njr, xpar)."""
    instrs = []
    for g in range(16):
        f_lo, f_hi = 144 * g, 144 * g + 144
        cuts = [f_lo] + [256 * k for k in range(1, 9) if f_lo < 256 * k < f_hi] + [f_hi]
        for a, b in zip(cuts, cuts[1:]):
            ki = a // 256
            for (gc0, ngc, j0, nj) in _rect_decomp(a - 144 * g, b - 144 * g):
                if ngc == 1:
                    halves = [(gc0, 1, 1)]
                else:
                    halves = [(gc0, 2, (ngc + 1) // 2), (gc0 + 1, 2, ngc // 2)]
                    halves = [h for h in halves if h[2] > 0]
                for (gcs, step, n) in halves:
                    ch0 = 144 * g + 9 * gcs + j0 - 256 * ki
                    # left-extend odd-j0 runs by one (duplicate of the j0-1
                    # product) ONLY when it stays within this tap's channels
                    ext = 1 if (j0 % 2 == 1 and ch0 >= 1) else 0
                    xpar = (ch0 - ext) % 2
                    instrs.append((g, ki, gcs, step, n, j0 - ext, nj + ext, xpar))
    return instrs


def _build_program(bias_zero=False):
    import concourse.bass as bass
    import concourse.tile as tile
    from concourse import bacc, mybir
    from concourse.masks import make_identity

    f32 = mybir.dt.float32
    bf16 = mybir.dt.bfloat16
    AP = bass.AP

    nc = bacc.Bacc(None, target_bir_lowering=False)
    xball_d = nc.dram_tensor("xball", [128, NCHUNK * 6 * 2562], bf16,
                             kind="ExternalInput")
    xcm_d = nc.dram_tensor("xcm", [128, NCHUNK * 2048], bf16, kind="ExternalInput")
    w12_d = nc.dram_tensor("w12", [2, 128, M144], bf16, kind="ExternalInput")
    brow_d = nc.dram_tensor("brow", [1, M144], bf16, kind="ExternalInput")
    out_d = nc.dram_tensor("out", [128, 32 * 256], bf16, kind="ExternalOutput")

    INSTRS = _build_instrs()

    with tile.TileContext(nc) as tc:
        with (
            tc.tile_pool(name="consts", bufs=1) as consts,
            tc.tile_pool(name="xb", bufs=1) as xbp,
            tc.tile_pool(name="xcm", bufs=1) as xcmp,
            tc.tile_pool(name="w16", bufs=1) as w16p,
            tc.tile_pool(name="prod", bufs=1) as prodp,
            tc.tile_pool(name="outs", bufs=1) as outsp,
            tc.tile_pool(name="ps2", bufs=3, space="PSUM") as ps2p,
            tc.tile_pool(name="psr", bufs=4, space="PSUM") as psrp,
        ):
            ident = consts.tile([128, 128], bf16)
            make_identity(nc, ident[:])
            # preload the ACT copy table set early (one tiny copy)
            nc.scalar.copy(out=ident[0:1, 0:2], in_=ident[0:1, 0:2])
            w12a = consts.tile([128, M144], bf16)
            w12b = consts.tile([128, M144], bf16)
            
            nc.sync.dma_start(out=w12a[:], in_=w12_d[0])
            nc.sync.dma_start(out=w12b[:], in_=w12_d[1])
            brow = consts.tile([1, M144], bf16)
            nc.sync.dma_start(out=brow[:], in_=brow_d[:])
            ones1 = consts.tile([1, 128], bf16)
            nc.gpsimd.memset(ones1[:], 1.0)

            pswu = pswup.tile([128, 128], f32)
            for _ in range(30):
                nc.tensor.matmul(pswu[:], lhsT=ident[:], rhs=ident[:],
                                 start=True, stop=True)
            def dma_xcm_chunk(c):
                t = xcmp.tile([128, 2048], bf16, name=f"xcm_{c % 2}")
                nc.sync.dma_start(
                    out=t[:],
                    in_=AP(xcm_d, c * 2048, [[NCHUNK * 2048, 128], [1, 2048]]))
                return t

            # x views: one packed per-chunk tile (6 views x 2562), 2-deep
            XNAMES = ("x0e", "x0o", "xue", "xuo", "xde", "xdo")
            XIDX = {n: i for i, n in enumerate(XNAMES)}
            XCH = 10 * 256 + 2

            def dma_x_chunk(c):
                t = xbp.tile([128, 6 * XCH], bf16, name=f"xpk_{c % 2}")
                half = 3 * XCH
                for h in range(4):
                    o0 = (h * half) // 2
                    o1 = ((h + 1) * half) // 2
                    nc.sync.dma_start(
                        out=AP(t.tensor, o0, [[6 * XCH, 128], [1, o1 - o0]]),
                        in_=AP(xball_d, c * 6 * XCH + o0,
                               [[NCHUNK * 6 * XCH, 128], [1, o1 - o0]]),
                    )
                return t

            def t2gen(c, xcmt, w16t):
                # 8 tiles: wl = 8c+t ; ps2[pix,144] = xcm_tile.T @ W12 + bias
                for t in range(WLC):
                    ps2 = ps2p.tile([128, M144], f32)
                    nc.tensor.matmul(
                        ps2[:],
                        lhsT=AP(xcmt.tensor, t * 128, [[2048, 128], [1, 128]]),
                        rhs=w12a[:], start=True, stop=False)
                    nc.tensor.matmul(
                        ps2[:],
                        lhsT=AP(xcmt.tensor, 1024 + t * 128, [[2048, 128], [1, 128]]),
                        rhs=w12b[:], start=False, stop=bias_zero)
                    if not bias_zero:
                        nc.tensor.matmul(ps2[:], lhsT=ones1[:], rhs=brow[:],
                                         start=False, stop=True)
                    # scatter t2[9g+j] -> W16[t*256 + 16g+j]
                    nc.scalar.copy(
                        out=AP(w16t.tensor, t * 256,
                               [[2048, 128], [16, 16], [1, 9]]),
                        in_=AP(ps2.tensor, 0, [[M144, 128], [9, 16], [1, 9]]),
                    )

            XNAME = {(-1, 0): "xde", (-1, 1): "xdo", (0, 0): "x0e",
                     (0, 1): "x0o", (1, 0): "xue", (1, 1): "xuo"}

            def _bases(g, ki, gcs, jlo, xpar):
                di, dj = ki // 3 - 1, ki % 3 - 1
                voff = XIDX[XNAME[(di, xpar)]] * XCH
                ch0 = 144 * g + 9 * gcs + jlo - 256 * ki + xpar + voff
                col = (16 * g + gcs) - (128 if g >= 8 else 0)
                return ((dj + 1) * 256 + ch0, 16 * g + jlo, col * 16 + jlo)

            # pair single-row instrs of matching (half, njr) via a 3rd AP dim
            MERGED = {False: [], True: []}
            for want_b in (False, True):
                sub = [i for i in INSTRS if (i[0] >= 8) == want_b]
                singles = {}
                for ins in sub:
                    if ins[4] == 1:
                        singles.setdefault(ins[6], []).append(ins)
                    else:
                        MERGED[want_b].append(("s", ins))
                for njr, lst in singles.items():
                    lst.sort(key=lambda i: _bases(i[0], i[1], i[2], i[5], i[7])[2])
                    while len(lst) >= 2:
                        i1, i2 = lst[0], lst[1]
                        b1 = _bases(i1[0], i1[1], i1[2], i1[5], i1[7])
                        b2 = _bases(i2[0], i2[1], i2[2], i2[5], i2[7])
                        if b2[0] > b1[0] and b2[1] > b1[1] and b2[2] > b1[2]:
                            MERGED[want_b].append(("p", i1, i2, b1, b2))
                            lst = lst[2:]
                        else:
                            MERGED[want_b].append(("s", lst.pop(0)))
                    for ins in lst:
                        MERGED[want_b].append(("s", ins))

            def mults(c, xt, w16t, Ma, Mb, want_b):
                    Mx = Mb if want_b else Ma
                    for item in MERGED[want_b]:
                        if item[0] == "s":
                            (g, ki, gcs, step, n, jlo, njr, xpar) = item[1]
                            xb0, wb0, ob0 = _bases(g, ki, gcs, jlo, xpar)
                            in_dims = [[6 * XCH, 128], [256, WLC]]
                            w_dims = [[2048, 128], [256, WLC]]
                            o_dims = [[PF // WLC * WLC, 128], [2048, WLC]]
                            if n > 1:
                                in_dims.append([9 * step, n])
                                w_dims.append([0, n])
                                o_dims.append([16 * step, n])
                            in_dims.append([1, njr])
                            w_dims.append([1, njr])
                            o_dims.append([1, njr])
                            nc.vector.tensor_mul(
                                AP(Mx.tensor, ob0, o_dims),
                                AP(xt.tensor, xb0, in_dims),
                                AP(w16t.tensor, wb0, w_dims),
                            )
                        else:
                            (_, i1, i2, b1, b2) = item
                            njr = i1[6]
                            nc.vector.tensor_mul(
                                AP(Mx.tensor, b1[2],
                                   [[PF, 128], [b2[2] - b1[2], 2],
                                    [2048, WLC], [1, njr]]),
                                AP(xt.tensor, b1[0],
                                   [[6 * XCH, 128], [b2[0] - b1[0], 2],
                                    [256, WLC], [1, njr]]),
                                AP(w16t.tensor, b1[1],
                                   [[2048, 128], [b2[1] - b1[1], 2],
                                    [256, WLC], [1, njr]]),
                            )

            def fold_pe(c, Mx, half, outc):
                # 2 groups of 512 (4 wl x 128 co); 9 slot-plane accum-MMs each
                for k in range(2):
                    psf = psfp.tile([128, 512], f32)
                    for s in range(9):
                        nc.tensor.matmul(
                            psf[:],
                            lhsT=ident[:],
                            rhs=AP(Mx.tensor, 4 * k * 2048 + s,
                                   [[PF, 128], [2048, 4], [16, 128]]),
                            start=(s == 0), stop=(s == 8))
                    # outc free layout: [wl(8), co(256)]; half b at col 128
                    nc.scalar.copy(
                        out=AP(outc.tensor, (4 * k) * 256 + half * 128,
                               [[2048, 128], [256, 4], [1, 128]]),
                        in_=AP(psf.tensor, 0, [[512, 128], [128, 4], [1, 128]]),
                    )

            def fold_dve(c, Mx, half, outc):
                # F1 in place: M[4..7] += M[0..3]
                nc.vector.tensor_add(
                    AP(Mx.tensor, 4, [[PF, 128], [2048, WLC], [16, 128], [1, 4]]),
                    AP(Mx.tensor, 0, [[PF, 128], [2048, WLC], [16, 128], [1, 4]]),
                    AP(Mx.tensor, 4, [[PF, 128], [2048, WLC], [16, 128], [1, 4]]),
                )
                # PE: psf = planes 4..8 per 512-group (4 wl x 128 co)
                for k in range(2):
                    psr = psrp.tile([128, 512], f32)
                    for si, sl in enumerate((4, 5, 6, 7, 8)):
                        nc.tensor.matmul(
                            psr[:], lhsT=ident[:],
                            rhs=AP(Mx.tensor, 4 * k * 2048 + sl,
                                   [[PF, 128], [2048, 4], [16, 128]]),
                            start=(si == 0), stop=(si == 4))
                    nc.scalar.copy(
                        out=AP(outc.tensor, (4 * k) * 256 + half * 128,
                               [[2048, 128], [256, 4], [1, 128]]),
                        in_=AP(psr.tensor, 0, [[512, 128], [128, 4], [1, 128]]),
                    )

            W16t = [w16p.tile([128, WLC * 256], bf16, name=f"w16_{i}") for i in range(4)]
            Ma = prodp.tile([128, PF], bf16)
            Mb = prodp.tile([128, PF], bf16)
            OUTt = [outsp.tile([128, WLC * 256], bf16, name=f"out_{i}") for i in range(2)]

            xcms = {0: dma_xcm_chunk(0)}
            xcms[1] = dma_xcm_chunk(1)
            xts = {0: dma_x_chunk(0), 1: dma_x_chunk(1)}
            t2gen(0, xcms.pop(0), W16t[0])
            for c in range(NCHUNK):
                xt = xts.pop(c)
                outc = OUTt[c % 2]
                mults(c, xt, W16t[c], Ma, Mb, False)
                fold_dve(c, Ma, 0, outc)
                nc.sync.dma_start(
                    out=AP(out_d, c * WLC * 256, [[32 * 256, 128], [256, WLC], [1, 128]]),
                    in_=AP(outc.tensor, 0, [[WLC * 256, 128], [256, WLC], [1, 128]]),
                )
                if c + 1 < NCHUNK:
                    t2gen(c + 1, xcms.pop(c + 1), W16t[c + 1])
                if c + 2 < NCHUNK:
                    xcms[c + 2] = dma_xcm_chunk(c + 2)
                mults(c, xt, W16t[c], Ma, Mb, True)
                fold_dve(c, Mb, 1, outc)
                if c + 2 < NCHUNK:
                    xts[c + 2] = dma_x_chunk(c + 2)
                nc.sync.dma_start(
                    out=AP(out_d, c * WLC * 256 + 128,
                           [[32 * 256, 128], [256, WLC], [1, 128]]),
                    in_=AP(outc.tensor, 128, [[WLC * 256, 128], [256, WLC], [1, 128]]),
                )
    nc.compile()
    return nc


def _get_program(bias_zero=False):
    key = ("nc", bias_zero)
    if key not in _cache:
        _cache[key] = _build_program(bias_zero)
    return _cache[key]


XCH2 = 10 * 256 + 2


def _host_prep(x, W1, b1, W2, b2):
    bf = ml_dtypes.bfloat16
    W12 = (W1.astype(np.float64) @ W2.astype(np.float64)).astype(np.float32)
    bfused = (b1.astype(np.float64) @ W2.astype(np.float64) + b2).astype(np.float32)
    w12_h = np.ascontiguousarray(W12.astype(bf).reshape(2, 128, M144))
    brow_h = np.ascontiguousarray(bfused.astype(bf).reshape(1, M144))

    in_maps = []
    for i in range(x.shape[0]):
        xi = x[i].astype(bf)  # [64, 64, 256]
        # pixel-major: [p=64*wq+h, slot, ch]; slot s <-> w = 32*wq + s - 1
        xpm4 = np.zeros((2, 64, SLOTS, 256), dtype=bf)
        xpm4[0, :, 1:34, :] = xi[:, 0:33, :]
        xpm4[1, :, 0:33, :] = xi[:, 31:64, :]
        xpm = np.zeros((128, XF), dtype=bf)
        xpm[:, :SLOTS * 256] = xpm4.reshape(128, SLOTS * 256)
        xu = np.zeros_like(xpm)   # row h+1
        xd = np.zeros_like(xpm)   # row h-1
        for wq in range(2):
            xu[64 * wq:64 * wq + 63] = xpm[64 * wq + 1:64 * wq + 64]
            xd[64 * wq + 1:64 * wq + 64] = xpm[64 * wq:64 * wq + 63]
        def odd(a):
            o = np.zeros_like(a)
            o[:, 1:] = a[:, :-1]
            return o
        # channel-major packed per chunk: [ch%128, c, ch//128, wl_t*128+p]
        xc = xi.reshape(64, 2, 32, 256).transpose(3, 2, 1, 0).reshape(2, 128, 32, 128)
        # xc[h, p, wl, pix]; want [p, chunk, h, t*128+pix]
        xcm = np.ascontiguousarray(
            xc.reshape(2, 128, 4, 8, 128).transpose(1, 2, 0, 3, 4).reshape(128, -1))
        views = [xpm, odd(xpm), xu, odd(xu), xd, odd(xd)]
        xball = np.zeros((128, NCHUNK, 6, XCH2), dtype=bf)
        for c in range(NCHUNK):
            for v, a in enumerate(views):
                xball[:, c, v, :] = a[:, WLC * c * 256:WLC * c * 256 + XCH2]
        in_maps.append({
            "xball": np.ascontiguousarray(xball.reshape(128, -1)),
            "xcm": xcm, "w12": w12_h, "brow": brow_h,
        })
    return in_maps


def kernel(x, W1, b1, W2, b2, trace=False):
    from concourse.bass_utils import run_bass_kernel_spmd

    bfused = np.asarray(b1, np.float64) @ np.asarray(W2, np.float64) + np.asarray(b2)
    nc = _get_program(bias_zero=not np.any(bfused))
    in_maps = _host_prep(np.asarray(x), W1, b1, W2, b2)
    res = run_bass_kernel_spmd(nc, in_maps, core_ids=list(range(NCORES)),
                               trace=trace)
    outs = []
    for i in range(NCORES):
        buf = np.asarray(res.results[i]["out"]).astype(np.float32)
        # [64*wq+h, wl*256+c] -> [h, w, c]
        o = buf.reshape(2, 64, 32, 256).transpose(1, 0, 2, 3).reshape(64, 64, 256)
        outs.append(o)
    out = np.stack(outs, axis=0)
    if trace:
        return out, res
    return out
